# revision 1
# baseline (speedup 1.0000x reference)
"""DoubleAttention TRN2 Bass kernel.

Full inputs in, full outputs out. Data-parallel over batch: B=32 split as
4 batches per core across 8 NeuronCores; 1x1-conv weights replicated.

Per-batch math (C = Cout = dn = 512, N = H*W = 1024):
  A   = wA @ x + bA            [C, N]
  smB = softmax(wB @ x, n)     (bB drops: softmax shift-invariant)
  smV = softmax(wV @ x, n)     (bV drops)
  G   = A @ smB^T              [C, C]
  Z   = wR @ (G @ smV) + bR    [C, N]

Kernel-side formulation (everything float32r on the PE):
  AT[n,c]   = sum_c' x[c',n] wA^T[c',c]        (transposed conv; no transposes
  EBT[n,d]  = exp(sum_c' x[c',n] wB^T[c',d])    needed for the n-contraction)
  EV[d,n]   = exp(Vm[d,n]); sV[d] = sum_n EV[d,n]   (natural layout)
  sB[d]     = sum_n EBT[n,d]    via ones-matmul + rank-1 transpose matmuls
  GrawT[d,c]= sum_n EBT[n,d] AT[n,c]
  GT[d,c]   = GrawT[d,c]/(sB[d]*sV[d]) + bA[c]/sV[d]   (scale+bias on evac)
  Z0[c,n]   = sum_d GT[d,c] EV[d,n]
  out[o,n]  = sum_c wR^T[c,o] Z0[c,n] + bR[o]   (bias via ACT activation)
"""

import numpy as np

B, C, N = 32, 512, 1024  # batch, channels, spatial (32*32)
H = W = 32
NCORES = 8
BPC = B // NCORES   # batches per core
KT = C // 128       # 4 contraction tiles
NT = N // 128       # 8 n-partition tiles
NS = N // 512       # 2 n free-dim spans

_CACHE = {}


def _build_nc():
    import concourse.bacc as bacc
    import concourse.mybir as mybir
    import concourse.tile as tile

    F32 = mybir.dt.float32
    F32R = mybir.dt.float32r
    AF = mybir.ActivationFunctionType

    nc = bacc.Bacc("TRN2", target_bir_lowering=False, debug=False,
                   num_devices=NCORES)
    x_d = nc.dram_tensor("x", [BPC, C, N], F32R, kind="ExternalInput").ap()
    wat_d = nc.dram_tensor("wat", [C, C], F32R, kind="ExternalInput").ap()
    wbt_d = nc.dram_tensor("wbt", [C, C], F32R, kind="ExternalInput").ap()
    wvt_d = nc.dram_tensor("wvt", [C, C], F32R, kind="ExternalInput").ap()
    wrt_d = nc.dram_tensor("wrt", [C, C], F32R, kind="ExternalInput").ap()
    bab_d = nc.dram_tensor("bab", [128, C], F32, kind="ExternalInput").ap()
    br_d = nc.dram_tensor("br", [128, KT], F32, kind="ExternalInput").ap()
    ones_d = nc.dram_tensor("ones", [128, 128], F32R, kind="ExternalInput").ap()
    o_d = nc.dram_tensor("o", [BPC, C, N], F32, kind="ExternalOutput").ap()

    with tile.TileContext(nc) as tc:
        with tc.tile_pool(name="wp", bufs=1) as wp, \
             tc.tile_pool(name="xp", bufs=2) as xp, \
             tc.tile_pool(name="ip", bufs=1) as ip, \
             tc.tile_pool(name="op", bufs=2) as op_, \
             tc.tile_pool(name="sp", bufs=2) as sp, \
             tc.tile_pool(name="pp", bufs=8, space="PSUM") as pp:

            wat = wp.tile([128, KT, C], F32R, tag="wat")
            wbt = wp.tile([128, KT, C], F32R, tag="wbt")
            wvt = wp.tile([128, KT, C], F32R, tag="wvt")
            wrt = wp.tile([128, KT, C], F32R, tag="wrt")
            xs0 = xp.tile([128, KT, N], F32R, tag="xs")
            ones = wp.tile([128, 128], F32R, tag="ones")
            # Warm the PE HAM clock gate during the DMA head: 4 slow fp32
            # matmuls (4 cyc/row) on a memset tile keep the array busy for
            # the ~3.4us SHORT window and finish before the real stream.
            garb = wp.tile([128, 512], F32, tag="garb")
            nc.gpsimd.memset(garb[:], 1.0)
            psw = pp.tile([128, 512], F32, tag="mm")
            for _ in range(2):
                nc.tensor.matmul(psw[:], garb[:, 0:128], garb[:],
                                 start=True, stop=True)
            # DMA priority order for batch 0: the first PV group needs
            # x[:, :, 0:512] plus wvt. Medium chunks on alternating queues
            # maximize early aggregate bandwidth without flooding the SP
            # sequencer with triggers.
            for k in range(KT):
                nc.sync.dma_start(xs0[:, k, 0:512],
                                  x_d[0, k * 128:(k + 1) * 128, 0:512])
                nc.sync.dma_start(wvt[:, k, :],
                                  wvt_d[k * 128:(k + 1) * 128, :])
            nc.sync.dma_start(xs0[:, :, 512:1024],
                              x_d[0, :, 512:1024].rearrange(
                                  "(k p) n -> p k n", p=128))
            for k in range(KT):
                nc.sync.dma_start(wat[:, k, :],
                                  wat_d[k * 128:(k + 1) * 128, :])
                nc.sync.dma_start(wbt[:, k, :],
                                  wbt_d[k * 128:(k + 1) * 128, :])
            nc.sync.dma_start(wrt[:], wrt_d.rearrange("(k p) c -> p k c",
                                                      p=128))
            nc.sync.dma_start(ones[:], ones_d[:])
            bab = wp.tile([128, C], F32, tag="bab")
            nc.sync.dma_start(bab[:], bab_d[:])
            br = wp.tile([128, KT], F32, tag="br")
            nc.sync.dma_start(br[:], br_d[:])

            for b in range(BPC):
                if b == 0:
                    xs = xs0
                else:
                    xs = xp.tile([128, KT, N], F32R, tag="xs")
                    for h in range(NS):
                        hsl = slice(h * 512, (h + 1) * 512)
                        nc.sync.dma_start(
                            xs[:, :, hsl],
                            x_d[b, :, hsl].rearrange("(k p) n -> p k n",
                                                     p=128))

                at = ip.tile([128, NT, C], F32R, tag="at")
                ebt = ip.tile([128, NT, C], F32R, tag="ebt")
                ev = ip.tile([128, KT, N], F32R, tag="ev")
                gt = ip.tile([128, KT, C], F32R, tag="gt")
                zs = ip.tile([128, KT, N], F32R, tag="zs")
                av = sp.tile([128, KT, NS], F32, tag="av")
                svc = sp.tile([128, KT], F32, tag="svc")
                sbc = sp.tile([128, KT], F32, tag="sbc")
                prod = sp.tile([128, KT], F32, tag="prod")
                rsc = sp.tile([128, KT], F32, tag="rsc")
                rsv = sp.tile([128, KT], F32, tag="rsv")
                sbr = sp.tile([1, C], F32R, tag="sbr")
                ebp = [sp.tile([128, C], F32R, tag=f"ebp{i}",
                               name=f"ebp{i}", bufs=1) for i in range(7)]
                os_ = op_.tile([128, KT, N], F32, tag="os")

                # Phase V: EV[d,n] natural + per-row expsums (h outer so
                # the first groups only need the first half of x)
                for h in range(NS):
                    hsl = slice(h * 512, (h + 1) * 512)
                    for dt in range(KT):
                        dsl = slice(dt * 128, (dt + 1) * 128)
                        psv = pp.tile([128, 512], F32, tag="mm")
                        for k in range(KT):
                            nc.tensor.matmul(psv[:], wvt[:, k, dsl],
                                             xs[:, k, hsl],
                                             start=(k == 0), stop=(k == KT - 1))
                        nc.scalar.activation(ev[:, dt, hsl], psv[:], AF.Exp,
                                             accum_out=av[:, dt, h:h + 1])
                nc.vector.tensor_add(svc[:], av[:, :, 0], av[:, :, 1])
                nc.vector.reciprocal(rsv[:], svc[:])

                # Phase 1: AT[n,c] and EBT[n,d] per n-tile.
                # k-paired order: consecutive matmuls share the same stationary
                # xs chunk (one weight set serves psa and psb).
                with nc.allow_low_precision(
                        reason="fp32r partials match the fp32r pipeline"):
                    for nt in range(NT):
                        nsl = slice(nt * 128, (nt + 1) * 128)
                        psa = pp.tile([128, C], F32, tag="mm")
                        psb = pp.tile([128, C], F32, tag="mm")
                        for k in range(KT):
                            nc.tensor.matmul(psa[:], xs[:, k, nsl],
                                             wat[:, k, :],
                                             start=(k == 0),
                                             stop=(k == KT - 1))
                            nc.tensor.matmul(psb[:], xs[:, k, nsl],
                                             wbt[:, k, :],
                                             start=(k == 0),
                                             stop=(k == KT - 1))
                        if nt == 7:
                            # exp + final tree add first: the sB chain is
                            # the critical path into pss; at-copy can wait
                            nc.scalar.activation(ebt[:, nt, :], psb[:],
                                                 AF.Exp)
                            nc.vector.tensor_add(ebp[6][:], ebp[5][:],
                                                 ebt[:, 7, :])
                            nc.vector.tensor_copy(at[:, nt, :], psa[:])
                            continue
                        nc.vector.tensor_copy(at[:, nt, :], psa[:])
                        nc.scalar.activation(ebt[:, nt, :], psb[:], AF.Exp)
                        # skewed sB partial-sum tree: only the nt=7 add
                        # depends on the last exp, so the pss matmul can
                        # issue almost immediately after P1.
                        if nt == 1:
                            nc.vector.tensor_add(ebp[0][:], ebt[:, 0, :],
                                                 ebt[:, 1, :])
                        elif nt == 3:
                            nc.vector.tensor_add(ebp[1][:], ebt[:, 2, :],
                                                 ebt[:, 3, :])
                            nc.vector.tensor_add(ebp[2][:], ebp[0][:],
                                                 ebp[1][:])
                        elif nt == 5:
                            nc.vector.tensor_add(ebp[3][:], ebt[:, 4, :],
                                                 ebt[:, 5, :])
                        elif nt == 6:
                            nc.vector.tensor_add(ebp[4][:], ebp[3][:],
                                                 ebt[:, 6, :])
                            nc.vector.tensor_add(ebp[5][:], ebp[2][:],
                                                 ebp[4][:])

                # Phase G: GrawT[d,c]; evac folds the 1/(sB*sV) scale and
                # the +bA[c] bias (GT = GrawT*rscale + bA_bcast*rsV). The sB
                # reduction (one ones-matmul over the tree total + K=1
                # row->column transpose matmuls) slots in after the first
                # group so its chain hides under the remaining groups.
                def gt_evac(dt, psg):
                    gta = sp.tile([128, C], F32, tag="gta", name="gta")
                    nc.scalar.mul(gta[:], psg[:], rsc[:, dt:dt + 1])
                    tmpb = sp.tile([128, C], F32, tag="tmpb", name="tmpb")
                    nc.vector.tensor_scalar_mul(tmpb[:], bab[:],
                                                rsv[:, dt:dt + 1])
                    nc.vector.tensor_add(gt[:, dt, :], gta[:], tmpb[:])

                # PE order: psg0, pss, psg1, psc, psg2, psg3 — the 1-lane
                # sbr row-copy (pss -> psc dependency) hides under psg1.
                psgs = []
                for dt in range(KT):
                    dsl = slice(dt * 128, (dt + 1) * 128)
                    psg = pp.tile([128, C], F32, tag="mm")
                    for nt in range(NT):
                        nc.tensor.matmul(psg[:], ebt[:, nt, dsl], at[:, nt, :],
                                         start=(nt == 0), stop=(nt == NT - 1))
                    if dt == 0:
                        psgs.append(psg)
                        pss = pp.tile([128, 512], F32, tag="mm")
                        nc.tensor.matmul(pss[:], ones[:], ebp[6][:],
                                         start=True, stop=True)
                        nc.vector.tensor_copy(sbr[:], pss[0:1, :])
                        continue
                    if dt == 1:
                        psgs.append(psg)
                        psc = pp.tile([128, KT, 2], F32, tag="mm")
                        for dtc in range(KT):
                            nc.tensor.matmul(
                                psc[:, dtc, :],
                                sbr[0:1, dtc * 128:(dtc + 1) * 128],
                                ones[0:1, 0:2], start=True, stop=True)
                        nc.vector.tensor_copy(sbc[:], psc[:, :, 0])
                        nc.vector.tensor_mul(prod[:], sbc[:], svc[:])
                        nc.vector.reciprocal(rsc[:], prod[:])
                        gt_evac(0, psgs[0])
                        gt_evac(1, psgs[1])
                        continue
                    gt_evac(dt, psg)

                # Phase Z: Z0[c,n]
                for ct in range(KT):
                    csl = slice(ct * 128, (ct + 1) * 128)
                    for h in range(NS):
                        hsl = slice(h * 512, (h + 1) * 512)
                        psz = pp.tile([128, 512], F32, tag="mm")
                        for dt in range(KT):
                            nc.tensor.matmul(psz[:], gt[:, dt, csl],
                                             ev[:, dt, hsl],
                                             start=(dt == 0), stop=(dt == KT - 1))
                        nc.vector.tensor_copy(zs[:, ct, hsl], psz[:])

                # Phase R: out[o,n] = wR @ Z + bR
                for ot in range(KT):
                    osl = slice(ot * 128, (ot + 1) * 128)
                    for h in range(NS):
                        hsl = slice(h * 512, (h + 1) * 512)
                        psr = pp.tile([128, 512], F32, tag="mm")
                        for k in range(KT):
                            nc.tensor.matmul(psr[:], wrt[:, k, osl],
                                             zs[:, k, hsl],
                                             start=(k == 0), stop=(k == KT - 1))
                        nc.scalar.activation(os_[:, ot, hsl], psr[:],
                                             AF.Identity, bias=br[:, ot:ot + 1])
                        nc.sync.dma_start(
                            o_d[b, ot * 128:(ot + 1) * 128, h * 512:(h + 1) * 512],
                            os_[:, ot, hsl])
    nc.compile()
    return nc


def _in_maps(x, wA, bA, wB, wV, wR, bR):
    xr = np.ascontiguousarray(x.reshape(B, C, N), dtype=np.float32)
    wat = np.ascontiguousarray(wA.T, dtype=np.float32)
    wbt = np.ascontiguousarray(wB.T, dtype=np.float32)
    wvt = np.ascontiguousarray(wV.T, dtype=np.float32)
    wrt = np.ascontiguousarray(wR.T, dtype=np.float32)
    bab = np.ascontiguousarray(
        np.broadcast_to(bA.reshape(1, C), (128, C)), dtype=np.float32)
    br = np.ascontiguousarray(bR.reshape(KT, 128).T, dtype=np.float32)
    ones = np.ones((128, 128), dtype=np.float32)
    maps = []
    for i in range(NCORES):
        maps.append({
            "x": np.ascontiguousarray(xr[i * BPC:(i + 1) * BPC]),
            "wat": wat, "wbt": wbt, "wvt": wvt, "wrt": wrt,
            "bab": bab, "br": br, "ones": ones,
        })
    return maps


def kernel(x, wA, bA, wB, bB, wV, bV, wR, bR):
    from concourse.bass_utils import run_bass_kernel_spmd
    if "nc" not in _CACHE:
        _CACHE["nc"] = _build_nc()
    nc = _CACHE["nc"]
    maps = _in_maps(x, wA, bA, wB, wV, wR, bR)
    res = run_bass_kernel_spmd(nc, maps, list(range(NCORES)))
    out = np.concatenate([res.results[i]["o"] for i in range(NCORES)], axis=0)
    return out.reshape(B, C, H, W).astype(np.float32)



# revision 2
# speedup vs baseline: 1.2309x; 1.2309x over previous
"""DoubleAttention TRN2 Bass kernel.

Full inputs in, full outputs out. Data-parallel over batch: B=32 split as
4 batches per core across 8 NeuronCores; weights replicated.

Reference math per batch (C = Cout = dn = 512, N = H*W = 1024):
  A   = wA @ x + bA            [C, N]
  smB = softmax(wB @ x, n)     (bB drops: softmax shift-invariant)
  smV = softmax(wV @ x, n)     (bV drops)
  G   = A @ smB^T              [C, C]
  Z   = wR @ (G @ smV) + bR    [C, N]

Restructured: softmax rows sum to 1, so G = wA (x smB^T) + bA 1^T and
  Z = WRA (x EB^T) diag(rsB rsV) EV + (wR bA) (rsV^T EV) + bR 1^T
with WRA = wR wA (host-precomputed), rsB = 1/rowsum(EB), rsV likewise.
This removes the A-projection entirely and shrinks the [C,C]x[C,C]
product to half a projection: 144 512-col matmuls/batch vs 192.

Kernel phases (everything float32r on the PE):
  V: EV[d,n]   = exp(wV x)      natural layout + row expsums    (32 mm)
  B: EBT[n,d]  = exp(x^T wB^T)  x chunks stationary             (32 mm)
  M: Mraw[c,d] = sum_n xT[n,c] EBT[n,d]   (xT DMA'd from host)  (32 mm)
  P: PT[d,o]   = sum_c Mraw[c,d] WRAT[c,o]; evac folds the
     1/(sB sV) scale and the +c[o]/sV[d] rank-1 term            (16 mm)
  F: out[o,n]  = sum_d PT[d,o] EV[d,n] + bR[o]                  (32 mm)
  sB[d] via skewed DVE partial-sum tree + ones-matmul + rank-1
  row->column transpose matmuls (as columns of [128,KT]).
"""

import numpy as np

B, C, N = 32, 512, 1024  # batch, channels, spatial (32*32)
H = W = 32
NCORES = 8
BPC = B // NCORES   # batches per core
KT = C // 128       # 4 contraction tiles
NT = N // 128       # 8 n-partition tiles
NS = N // 512       # 2 n free-dim spans

_CACHE = {}


def _build_nc():
    import concourse.bacc as bacc
    import concourse.mybir as mybir
    import concourse.tile as tile
    from concourse.alu_op_type import AluOpType

    F32 = mybir.dt.float32
    F32R = mybir.dt.float32r
    AF = mybir.ActivationFunctionType

    nc = bacc.Bacc("TRN2", target_bir_lowering=False, debug=False,
                   num_devices=NCORES)
    x_d = nc.dram_tensor("x", [BPC, C, N], F32R, kind="ExternalInput").ap()
    xt_d = nc.dram_tensor("xt", [BPC, N, C], F32R, kind="ExternalInput").ap()
    wbt_d = nc.dram_tensor("wbt", [C, C], F32R, kind="ExternalInput").ap()
    wvt_d = nc.dram_tensor("wvt", [C, C], F32R, kind="ExternalInput").ap()
    wrat_d = nc.dram_tensor("wrat", [C, C], F32R, kind="ExternalInput").ap()
    cb_d = nc.dram_tensor("cb", [128, C], F32, kind="ExternalInput").ap()
    br_d = nc.dram_tensor("br", [128, KT], F32, kind="ExternalInput").ap()
    ones_d = nc.dram_tensor("ones", [128, 128], F32R, kind="ExternalInput").ap()
    o_d = nc.dram_tensor("o", [BPC, C, N], F32, kind="ExternalOutput").ap()

    with tile.TileContext(nc) as tc:
        with tc.tile_pool(name="wp", bufs=1) as wp, \
             tc.tile_pool(name="xp", bufs=2) as xp, \
             tc.tile_pool(name="ip", bufs=1) as ip, \
             tc.tile_pool(name="op", bufs=1) as op_, \
             tc.tile_pool(name="sp", bufs=2) as sp, \
             tc.tile_pool(name="pp", bufs=8, space="PSUM") as pp:

            wbt = wp.tile([128, KT, C], F32R, tag="wbt")
            wvt = wp.tile([128, KT, C], F32R, tag="wvt")
            wrat = wp.tile([128, KT, C], F32R, tag="wrat")
            xs0 = xp.tile([128, KT, N], F32R, tag="xs")
            xt0 = xp.tile([128, NT, C], F32R, tag="xt")
            ones = wp.tile([128, 128], F32R, tag="ones")
            # Warm the PE HAM clock gate during the DMA head: slow fp32
            # matmuls (4 cyc/row) on a memset tile keep the array busy
            # through the cold-clock window and finish before the stream.
            garb = wp.tile([128, 512], F32, tag="garb")
            nc.gpsimd.memset(garb[:], 1.0)
            psw = pp.tile([128, 512], F32, tag="mm")
            for _ in range(2):
                nc.tensor.matmul(psw[:], garb[:, 0:128], garb[:],
                                 start=True, stop=True)
            # DMA priority order for batch 0: the first V groups need
            # wvt plus x[:, :, 0:512]; B needs wbt by ~7us, M needs xt
            # by ~14us, P needs wrat by ~21us.
            for k in range(KT):
                nc.sync.dma_start(wvt[:, k, :],
                                  wvt_d[k * 128:(k + 1) * 128, :])
                nc.sync.dma_start(xs0[:, k, 0:512],
                                  x_d[0, k * 128:(k + 1) * 128, 0:512])
            nc.sync.dma_start(xs0[:, :, 512:1024],
                              x_d[0, :, 512:1024].rearrange(
                                  "(k p) n -> p k n", p=128))
            for k in range(KT):
                nc.sync.dma_start(wbt[:, k, :],
                                  wbt_d[k * 128:(k + 1) * 128, :])
            nc.sync.dma_start(ones[:], ones_d[:])
            cb = wp.tile([128, C], F32, tag="cb")
            nc.sync.dma_start(cb[:], cb_d[:])
            br = wp.tile([128, KT], F32, tag="br")
            nc.sync.dma_start(br[:], br_d[:])
            for h in range(NS):
                nc.sync.dma_start(
                    xt0[:, h * KT:(h + 1) * KT, :],
                    xt_d[0, h * 512:(h + 1) * 512, :].rearrange(
                        "(t p) c -> p t c", p=128))
            nc.sync.dma_start(wrat[:], wrat_d.rearrange("(k p) c -> p k c",
                                                        p=128))

            for b in range(BPC):
                if b == 0:
                    xs, xt = xs0, xt0
                else:
                    xs = xp.tile([128, KT, N], F32R, tag="xs")
                    xt = xp.tile([128, NT, C], F32R, tag="xt")
                    for h in range(NS):
                        hsl = slice(h * 512, (h + 1) * 512)
                        nc.sync.dma_start(
                            xs[:, :, hsl],
                            x_d[b, :, hsl].rearrange("(k p) n -> p k n",
                                                     p=128))
                        nc.sync.dma_start(
                            xt[:, h * KT:(h + 1) * KT, :],
                            xt_d[b, hsl, :].rearrange("(t p) c -> p t c",
                                                      p=128))

                ebt = ip.tile([128, NT, C], F32R, tag="ebt")
                ev = ip.tile([128, KT, N], F32R, tag="ev")
                m_ = ip.tile([128, KT, C], F32R, tag="m")
                pt_ = ip.tile([128, KT, C], F32R, tag="pt")
                av = sp.tile([128, KT, NS], F32, tag="av")
                svc = sp.tile([128, KT], F32, tag="svc")
                sbc = sp.tile([128, KT], F32, tag="sbc")
                prod = sp.tile([128, KT], F32, tag="prod")
                rsc = sp.tile([128, KT], F32, tag="rsc")
                rsv = sp.tile([128, KT], F32, tag="rsv")
                tb = sp.tile([128, KT, C], F32, tag="tb")
                sbr = sp.tile([1, C], F32R, tag="sbr")
                ebp = [sp.tile([128, C], F32R, tag=f"ebp{i}",
                               name=f"ebp{i}", bufs=1) for i in range(7)]
                os_ = op_.tile([128, KT, N], F32, tag="os")

                # Phase V: EV[d,n] natural + per-row expsums (h outer so
                # the first groups only need the first half of x)
                with nc.named_scope(f"V{b}"):
                    for h in range(NS):
                        hsl = slice(h * 512, (h + 1) * 512)
                        for dt in range(KT):
                            dsl = slice(dt * 128, (dt + 1) * 128)
                            psv = pp.tile([128, 512], F32, tag="mm")
                            for k in range(KT):
                                nc.tensor.matmul(psv[:], wvt[:, k, dsl],
                                                 xs[:, k, hsl],
                                                 start=(k == 0),
                                                 stop=(k == KT - 1))
                            nc.scalar.activation(ev[:, dt, hsl], psv[:],
                                                 AF.Exp,
                                                 accum_out=av[:, dt, h:h + 1])
                    nc.vector.tensor_add(svc[:], av[:, :, 0], av[:, :, 1])
                    nc.vector.reciprocal(rsv[:], svc[:])
                    # tb[p,o] = c[o] * rsV[dt-chunk p] — the rank-1 term of
                    # the P evac; hoisted here (only needs rsv, not psp).
                    for dt in range(KT):
                        nc.vector.tensor_scalar_mul(tb[:, dt, :], cb[:],
                                                    rsv[:, dt:dt + 1])

                # Phase B: EBT[n,d] per n-tile; skewed sB partial-sum
                # tree: only the nt=7 add depends on the last exp, so the
                # pss matmul can issue almost immediately after B.
                with nc.named_scope(f"B{b}"), nc.allow_low_precision(
                        reason="fp32r partials match the fp32r pipeline"):
                    for nt in range(NT):
                        nsl = slice(nt * 128, (nt + 1) * 128)
                        psb = pp.tile([128, C], F32, tag="mm")
                        for k in range(KT):
                            nc.tensor.matmul(psb[:], xs[:, k, nsl],
                                             wbt[:, k, :],
                                             start=(k == 0),
                                             stop=(k == KT - 1))
                        nc.scalar.activation(ebt[:, nt, :], psb[:], AF.Exp)
                        if nt == 1:
                            nc.vector.tensor_add(ebp[0][:], ebt[:, 0, :],
                                                 ebt[:, 1, :])
                        elif nt == 3:
                            nc.vector.tensor_add(ebp[1][:], ebt[:, 2, :],
                                                 ebt[:, 3, :])
                            nc.vector.tensor_add(ebp[2][:], ebp[0][:],
                                                 ebp[1][:])
                        elif nt == 5:
                            nc.vector.tensor_add(ebp[3][:], ebt[:, 4, :],
                                                 ebt[:, 5, :])
                        elif nt == 6:
                            nc.vector.tensor_add(ebp[4][:], ebp[3][:],
                                                 ebt[:, 6, :])
                            nc.vector.tensor_add(ebp[5][:], ebp[2][:],
                                                 ebp[4][:])
                        elif nt == 7:
                            nc.vector.tensor_add(ebp[6][:], ebp[5][:],
                                                 ebt[:, 7, :])

                # Phase M: Mraw[c,d] = sum_n xT[n,c] EBT[n,d]. The sB
                # reduction (ones-matmul over the tree total + K=1
                # row->column transpose matmuls) slots in after the first
                # groups so its chain hides under the remaining groups.
                with nc.named_scope(f"M{b}"):
                    for ct in range(KT):
                        csl = slice(ct * 128, (ct + 1) * 128)
                        psm = pp.tile([128, C], F32, tag="mm")
                        for nt in range(NT):
                            nc.tensor.matmul(psm[:], xt[:, nt, csl],
                                             ebt[:, nt, :],
                                             start=(nt == 0),
                                             stop=(nt == NT - 1))
                        nc.scalar.copy(m_[:, ct, :], psm[:])
                        if ct == 0:
                            pss = pp.tile([128, 512], F32, tag="mm")
                            nc.tensor.matmul(pss[:], ones[:], ebp[6][:],
                                             start=True, stop=True)
                            nc.vector.tensor_copy(sbr[:], pss[0:1, :])
                        elif ct == 1:
                            psc = pp.tile([128, KT, 2], F32, tag="mm")
                            for dtc in range(KT):
                                nc.tensor.matmul(
                                    psc[:, dtc, :],
                                    sbr[0:1, dtc * 128:(dtc + 1) * 128],
                                    ones[0:1, 0:2], start=True, stop=True)
                            nc.vector.tensor_copy(sbc[:], psc[:, :, 0])
                            nc.vector.tensor_mul(prod[:], sbc[:], svc[:])
                            nc.vector.reciprocal(rsc[:], prod[:])

                # Phase P: PT[d,o]; single fused evac op folds the
                # 1/(sB*sV) scale and adds the hoisted rank-1 term.
                with nc.named_scope(f"P{b}"):
                    for dt in range(KT):
                        dsl = slice(dt * 128, (dt + 1) * 128)
                        psp = pp.tile([128, C], F32, tag="mm")
                        for ct in range(KT):
                            nc.tensor.matmul(psp[:], m_[:, ct, dsl],
                                             wrat[:, ct, :],
                                             start=(ct == 0),
                                             stop=(ct == KT - 1))
                        nc.vector.scalar_tensor_tensor(
                            pt_[:, dt, :], psp[:], rsc[:, dt:dt + 1],
                            tb[:, dt, :], op0=AluOpType.mult,
                            op1=AluOpType.add)

                # Phase F: out[o,n] = PT^T EV + bR (bias via ACT), DMA out
                with nc.named_scope(f"F{b}"):
                    for ot in range(KT):
                        osl = slice(ot * 128, (ot + 1) * 128)
                        for h in range(NS):
                            hsl = slice(h * 512, (h + 1) * 512)
                            psf = pp.tile([128, 512], F32, tag="mm")
                            for dt in range(KT):
                                nc.tensor.matmul(psf[:], pt_[:, dt, osl],
                                                 ev[:, dt, hsl],
                                                 start=(dt == 0),
                                                 stop=(dt == KT - 1))
                            nc.scalar.activation(os_[:, ot, hsl], psf[:],
                                                 AF.Identity,
                                                 bias=br[:, ot:ot + 1])
                            nc.sync.dma_start(
                                o_d[b, ot * 128:(ot + 1) * 128,
                                    h * 512:(h + 1) * 512],
                                os_[:, ot, hsl])
    nc.compile()
    return nc


def _in_maps(x, wA, bA, wB, wV, wR, bR):
    xr = np.ascontiguousarray(x.reshape(B, C, N), dtype=np.float32)
    xtr = np.ascontiguousarray(xr.transpose(0, 2, 1))
    wbt = np.ascontiguousarray(wB.T, dtype=np.float32)
    wvt = np.ascontiguousarray(wV.T, dtype=np.float32)
    wrat = np.ascontiguousarray((wR @ wA).T, dtype=np.float32)
    cvec = (wR @ bA).astype(np.float32)
    cb = np.ascontiguousarray(
        np.broadcast_to(cvec.reshape(1, C), (128, C)), dtype=np.float32)
    br = np.ascontiguousarray(bR.reshape(KT, 128).T, dtype=np.float32)
    ones = np.ones((128, 128), dtype=np.float32)
    maps = []
    for i in range(NCORES):
        maps.append({
            "x": np.ascontiguousarray(xr[i * BPC:(i + 1) * BPC]),
            "xt": np.ascontiguousarray(xtr[i * BPC:(i + 1) * BPC]),
            "wbt": wbt, "wvt": wvt, "wrat": wrat,
            "cb": cb, "br": br, "ones": ones,
        })
    return maps


def kernel(x, wA, bA, wB, bB, wV, bV, wR, bR):
    from concourse.bass_utils import run_bass_kernel_spmd
    if "nc" not in _CACHE:
        _CACHE["nc"] = _build_nc()
    nc = _CACHE["nc"]
    maps = _in_maps(x, wA, bA, wB, wV, wR, bR)
    res = run_bass_kernel_spmd(nc, maps, list(range(NCORES)))
    out = np.concatenate([res.results[i]["o"] for i in range(NCORES)], axis=0)
    return out.reshape(B, C, H, W).astype(np.float32)


# revision 16
# speedup vs baseline: 1.3429x; 1.0910x over previous
"""DoubleAttention TRN2 Bass kernel.

Full inputs in, full outputs out. Data-parallel over batch: B=32 split as
4 batches per core across 8 NeuronCores; weights replicated.

Reference math per batch (C = Cout = dn = 512, N = H*W = 1024):
  A   = wA @ x + bA            [C, N]
  smB = softmax(wB @ x, n)     (bB drops: softmax shift-invariant)
  smV = softmax(wV @ x, n)     (bV drops)
  G   = A @ smB^T              [C, C]
  Z   = wR @ (G @ smV) + bR    [C, N]

Restructured: softmax rows sum to 1, so G = wA (x smB^T) + bA 1^T and
  Z = WRA (x EB^T) diag(rsB rsV) EV + (wR bA) (rsV^T EV) + bR 1^T
with WRA = wR wA (host-precomputed), rsB = 1/rowsum(EB), rsV likewise.
This removes the A-projection entirely and shrinks the [C,C]x[C,C]
product to half a projection: 144 512-col matmuls/batch vs 192.

Kernel phases (everything float32r on the PE):
  V: EV[d,n]   = exp(wV x)      natural layout + row expsums    (32 mm)
  B: EBT[n,d]  = exp(x^T wB^T)  x chunks stationary             (32 mm)
  M: Mraw[c,d] = sum_n xT[n,c] EBT[n,d]   (xT DMA'd from host)  (32 mm)
  P: PT[d,o]   = sum_c Mraw[c,d] WRAT[c,o]; evac folds the
     1/(sB sV) scale and the +c[o]/sV[d] rank-1 term            (16 mm)
  F: out[o,n]  = sum_d PT[d,o] EV[d,n] + bR[o]                  (32 mm)
  sB[d] via skewed DVE partial-sum tree + ones-matmul + rank-1
  row->column transpose matmuls (as columns of [128,KT]).

x/xT and the projection weights stream in as bf16 (matmul speed is the
same 1 cyc/row, but the DMA head halves — batch 0's V phase was DMA
starved in fp32 and the resulting PE gaps held the clock at the mid
p-state). The P/F chain stays float32r end-to-end; PSUM is fp32.
"""

import numpy as np

B, C, N = 32, 512, 1024  # batch, channels, spatial (32*32)
H = W = 32
NCORES = 8
BPC = B // NCORES   # batches per core
KT = C // 128       # 4 contraction tiles
NT = N // 128       # 8 n-partition tiles
NS = N // 512       # 2 n free-dim spans

_CACHE = {}


def _build_nc():
    import concourse.bacc as bacc
    import concourse.mybir as mybir
    import concourse.tile as tile
    from concourse.alu_op_type import AluOpType

    F32 = mybir.dt.float32
    F32R = mybir.dt.float32r
    BF16 = mybir.dt.bfloat16
    AF = mybir.ActivationFunctionType

    nc = bacc.Bacc("TRN2", target_bir_lowering=False, debug=False,
                   num_devices=NCORES)
    x_d = nc.dram_tensor("x", [BPC, C, N], BF16, kind="ExternalInput").ap()
    xt_d = nc.dram_tensor("xt", [BPC, N, C], BF16, kind="ExternalInput").ap()
    wbt_d = nc.dram_tensor("wbt", [C, C], BF16, kind="ExternalInput").ap()
    wvt_d = nc.dram_tensor("wvt", [C, C], BF16, kind="ExternalInput").ap()
    wrat_d = nc.dram_tensor("wrat", [C, C], F32R, kind="ExternalInput").ap()
    cb_d = nc.dram_tensor("cb", [128, C], F32, kind="ExternalInput").ap()
    br_d = nc.dram_tensor("br", [128, KT], F32, kind="ExternalInput").ap()
    ones_d = nc.dram_tensor("ones", [128, 128], F32R, kind="ExternalInput").ap()
    o_d = nc.dram_tensor("o", [BPC, C, N], F32, kind="ExternalOutput").ap()

    with tile.TileContext(nc) as tc:
        with tc.tile_pool(name="wp", bufs=1) as wp, \
             tc.tile_pool(name="xp", bufs=2) as xp, \
             tc.tile_pool(name="ip", bufs=1) as ip, \
             tc.tile_pool(name="op", bufs=1) as op_, \
             tc.tile_pool(name="sp", bufs=2) as sp, \
             tc.tile_pool(name="pp", bufs=8, space="PSUM") as pp:

            wbt = wp.tile([128, KT, C], BF16, tag="wbt")
            wvt = wp.tile([128, KT, C], BF16, tag="wvt")
            wrat = wp.tile([128, KT, C], F32R, tag="wrat")
            xs0 = xp.tile([128, KT, N], BF16, tag="xs")
            xt0 = xp.tile([128, NT, C], BF16, tag="xt")
            ones = wp.tile([128, 128], F32R, tag="ones")
            # Warm the PE HAM clock gate during the DMA head: slow fp32
            # matmuls (4 cyc/row) on a memset tile keep the array busy
            # through the cold-clock window and finish before the stream.
            garb = wp.tile([128, 512], F32, tag="garb")
            nc.gpsimd.memset(garb[:], 1.0)
            psw = pp.tile([128, 512], F32, tag="mm")
            for _ in range(2):
                nc.tensor.matmul(psw[:], garb[:, 0:128], garb[:],
                                 start=True, stop=True)
            # DMA priority order for batch 0: the first V groups need
            # wvt plus x[:, :, 0:512]; B needs wbt by ~7us, M needs xt
            # by ~14us, P needs wrat by ~21us.
            for k in range(KT):
                nc.sync.dma_start(wvt[:, k, :],
                                  wvt_d[k * 128:(k + 1) * 128, :])
                nc.sync.dma_start(xs0[:, k, 0:512],
                                  x_d[0, k * 128:(k + 1) * 128, 0:512])
            nc.sync.dma_start(xs0[:, :, 512:1024],
                              x_d[0, :, 512:1024].rearrange(
                                  "(k p) n -> p k n", p=128))
            for k in range(KT):
                nc.sync.dma_start(wbt[:, k, :],
                                  wbt_d[k * 128:(k + 1) * 128, :])
            nc.sync.dma_start(ones[:], ones_d[:])
            cb = wp.tile([128, C], F32, tag="cb")
            nc.sync.dma_start(cb[:], cb_d[:])
            br = wp.tile([128, KT], F32, tag="br")
            nc.sync.dma_start(br[:], br_d[:])
            for h in range(NS):
                nc.sync.dma_start(
                    xt0[:, h * KT:(h + 1) * KT, :],
                    xt_d[0, h * 512:(h + 1) * 512, :].rearrange(
                        "(t p) c -> p t c", p=128))
            nc.sync.dma_start(wrat[:], wrat_d.rearrange("(k p) c -> p k c",
                                                        p=128))

            for b in range(BPC):
                if b == 0:
                    xs, xt = xs0, xt0
                else:
                    xs = xp.tile([128, KT, N], BF16, tag="xs")
                    xt = xp.tile([128, NT, C], BF16, tag="xt")
                    for h in range(NS):
                        hsl = slice(h * 512, (h + 1) * 512)
                        nc.sync.dma_start(
                            xs[:, :, hsl],
                            x_d[b, :, hsl].rearrange("(k p) n -> p k n",
                                                     p=128))
                        nc.sync.dma_start(
                            xt[:, h * KT:(h + 1) * KT, :],
                            xt_d[b, hsl, :].rearrange("(t p) c -> p t c",
                                                      p=128))

                ebt = ip.tile([128, NT, C], BF16, tag="ebt")
                ev = ip.tile([128, KT, N], F32R, tag="ev")
                m_ = ip.tile([128, KT, C], F32R, tag="m")
                pt_ = ip.tile([128, KT, C], F32R, tag="pt")
                av = sp.tile([128, KT, NS], F32, tag="av")
                svc = sp.tile([128, KT], F32, tag="svc")
                sbc = sp.tile([128, KT], F32, tag="sbc")
                prod = sp.tile([128, KT], F32, tag="prod")
                rsc = sp.tile([128, KT], F32, tag="rsc")
                rsv = sp.tile([128, KT], F32, tag="rsv")
                tb = sp.tile([128, KT, C], F32, tag="tb")
                sbr = sp.tile([1, C], F32R, tag="sbr")
                ebp = [sp.tile([128, C], F32R, tag=f"ebp{i}",
                               name=f"ebp{i}", bufs=1) for i in range(7)]
                os_ = op_.tile([128, KT, N], F32, tag="os")

                # Phase V: EV[d,n] natural + per-row expsums (h outer so
                # the first groups only need the first half of x)
                with nc.named_scope(f"V{b}"):
                    for h in range(NS):
                        hsl = slice(h * 512, (h + 1) * 512)
                        for dt in range(KT):
                            dsl = slice(dt * 128, (dt + 1) * 128)
                            psv = pp.tile([128, 512], F32, tag="mm")
                            for k in range(KT):
                                nc.tensor.matmul(psv[:], wvt[:, k, dsl],
                                                 xs[:, k, hsl],
                                                 start=(k == 0),
                                                 stop=(k == KT - 1))
                            nc.scalar.activation(ev[:, dt, hsl], psv[:],
                                                 AF.Exp,
                                                 accum_out=av[:, dt, h:h + 1])
                    nc.vector.tensor_add(svc[:], av[:, :, 0], av[:, :, 1])
                    nc.vector.reciprocal(rsv[:], svc[:])
                    # tb[p,o] = c[o] * rsV[dt-chunk p] — the rank-1 term of
                    # the P evac; hoisted here (only needs rsv, not psp).
                    for dt in range(KT):
                        nc.vector.tensor_scalar_mul(tb[:, dt, :], cb[:],
                                                    rsv[:, dt:dt + 1])

                # Phase B: EBT[n,d] per n-tile; skewed sB partial-sum
                # tree: only the nt=7 add depends on the last exp, so the
                # pss matmul can issue almost immediately after B.
                with nc.named_scope(f"B{b}"), nc.allow_low_precision(
                        reason="fp32r partials match the fp32r pipeline"):
                    for nt in range(NT):
                        nsl = slice(nt * 128, (nt + 1) * 128)
                        psb = pp.tile([128, C], F32, tag="mm")
                        for k in range(KT):
                            nc.tensor.matmul(psb[:], xs[:, k, nsl],
                                             wbt[:, k, :],
                                             start=(k == 0),
                                             stop=(k == KT - 1))
                        nc.scalar.activation(ebt[:, nt, :], psb[:], AF.Exp)
                        if nt == 1:
                            nc.vector.tensor_add(ebp[0][:], ebt[:, 0, :],
                                                 ebt[:, 1, :])
                        elif nt == 3:
                            nc.vector.tensor_add(ebp[1][:], ebt[:, 2, :],
                                                 ebt[:, 3, :])
                            nc.vector.tensor_add(ebp[2][:], ebp[0][:],
                                                 ebp[1][:])
                        elif nt == 5:
                            nc.vector.tensor_add(ebp[3][:], ebt[:, 4, :],
                                                 ebt[:, 5, :])
                        elif nt == 6:
                            nc.vector.tensor_add(ebp[4][:], ebp[3][:],
                                                 ebt[:, 6, :])
                            nc.vector.tensor_add(ebp[5][:], ebp[2][:],
                                                 ebp[4][:])
                        elif nt == 7:
                            nc.vector.tensor_add(ebp[6][:], ebp[5][:],
                                                 ebt[:, 7, :])

                # Phase M: Mraw[c,d] = sum_n xT[n,c] EBT[n,d]. The sB
                # reduction interleaves so each PE step's wait is already
                # satisfied when the in-order stream reaches it: the
                # ones-matmul after ct=1 (DVE tree total done by then),
                # the row->column transpose matmuls after ct=3 (the
                # 1-lane sbr copy done during ct=2/3); the DVE chain to
                # rsc then hides under the P matmuls.
                with nc.named_scope(f"M{b}"):
                    for ct in range(KT):
                        csl = slice(ct * 128, (ct + 1) * 128)
                        psm = pp.tile([128, C], F32, tag="mm")
                        for nt in range(NT):
                            nc.tensor.matmul(psm[:], xt[:, nt, csl],
                                             ebt[:, nt, :],
                                             start=(nt == 0),
                                             stop=(nt == NT - 1))
                        nc.scalar.copy(m_[:, ct, :], psm[:])
                        if ct == 1:
                            pss = pp.tile([128, 512], F32, tag="mm")
                            nc.tensor.matmul(pss[:], ones[:], ebp[6][:],
                                             start=True, stop=True)
                            nc.vector.tensor_copy(sbr[:], pss[0:1, :])
                    psc = pp.tile([128, KT, 2], F32, tag="mm")
                    for dtc in range(KT):
                        nc.tensor.matmul(
                            psc[:, dtc, :],
                            sbr[0:1, dtc * 128:(dtc + 1) * 128],
                            ones[0:1, 0:2], start=True, stop=True)
                    nc.vector.tensor_copy(sbc[:], psc[:, :, 0])
                    nc.vector.tensor_mul(prod[:], sbc[:], svc[:])
                    nc.vector.reciprocal(rsc[:], prod[:])

                # Phase P: PT[d,o]; single fused evac op folds the
                # 1/(sB*sV) scale and adds the hoisted rank-1 term.
                with nc.named_scope(f"P{b}"):
                    for dt in range(KT):
                        dsl = slice(dt * 128, (dt + 1) * 128)
                        psp = pp.tile([128, C], F32, tag="mm")
                        for ct in range(KT):
                            nc.tensor.matmul(psp[:], m_[:, ct, dsl],
                                             wrat[:, ct, :],
                                             start=(ct == 0),
                                             stop=(ct == KT - 1))
                        nc.vector.scalar_tensor_tensor(
                            pt_[:, dt, :], psp[:], rsc[:, dt:dt + 1],
                            tb[:, dt, :], op0=AluOpType.mult,
                            op1=AluOpType.add)

                # Phase F: out[o,n] = PT^T EV + bR (bias via ACT), DMA out
                with nc.named_scope(f"F{b}"):
                    for ot in range(KT):
                        osl = slice(ot * 128, (ot + 1) * 128)
                        for h in range(NS):
                            hsl = slice(h * 512, (h + 1) * 512)
                            psf = pp.tile([128, 512], F32, tag="mm")
                            for dt in range(KT):
                                nc.tensor.matmul(psf[:], pt_[:, dt, osl],
                                                 ev[:, dt, hsl],
                                                 start=(dt == 0),
                                                 stop=(dt == KT - 1))
                            nc.scalar.activation(os_[:, ot, hsl], psf[:],
                                                 AF.Identity,
                                                 bias=br[:, ot:ot + 1])
                            nc.sync.dma_start(
                                o_d[b, ot * 128:(ot + 1) * 128,
                                    h * 512:(h + 1) * 512],
                                os_[:, ot, hsl])
    nc.compile()
    return nc


def _in_maps(x, wA, bA, wB, wV, wR, bR):
    import ml_dtypes
    bf16 = ml_dtypes.bfloat16
    xr = x.reshape(B, C, N).astype(bf16)
    xtr = np.ascontiguousarray(xr.transpose(0, 2, 1))
    wbt = np.ascontiguousarray(wB.T.astype(bf16))
    wvt = np.ascontiguousarray(wV.T.astype(bf16))
    wrat = np.ascontiguousarray((wR @ wA).T, dtype=np.float32)
    cvec = (wR @ bA).astype(np.float32)
    cb = np.ascontiguousarray(
        np.broadcast_to(cvec.reshape(1, C), (128, C)), dtype=np.float32)
    br = np.ascontiguousarray(bR.reshape(KT, 128).T, dtype=np.float32)
    ones = np.ones((128, 128), dtype=np.float32)
    maps = []
    for i in range(NCORES):
        maps.append({
            "x": np.ascontiguousarray(xr[i * BPC:(i + 1) * BPC]),
            "xt": np.ascontiguousarray(xtr[i * BPC:(i + 1) * BPC]),
            "wbt": wbt, "wvt": wvt, "wrat": wrat,
            "cb": cb, "br": br, "ones": ones,
        })
    return maps


def kernel(x, wA, bA, wB, bB, wV, bV, wR, bR):
    from concourse.bass_utils import run_bass_kernel_spmd
    if "nc" not in _CACHE:
        _CACHE["nc"] = _build_nc()
    nc = _CACHE["nc"]
    maps = _in_maps(x, wA, bA, wB, wV, wR, bR)
    res = run_bass_kernel_spmd(nc, maps, list(range(NCORES)))
    out = np.concatenate([res.results[i]["o"] for i in range(NCORES)], axis=0)
    return out.reshape(B, C, H, W).astype(np.float32)
